# revision 13
# baseline (speedup 1.0000x reference)
"""Multi-head-free attention kernel for Trainium2, SPMD across 8 NeuronCores.

Problem: x[4, 4096, 512] -> Q,K,V = x@W* + b* (d_head=64);
Z = softmax(Q K^T / 8) V  -> [4, 4096, 64]

Sharding: data-parallel over batch (4) x query-halves (2) = 8 cores.
Each core handles 2048 queries of one batch against all 4096 keys of
that batch.  The key/value rows are fed in rolled order so every core's
queries sit at rows 0..2047 of its input -- softmax(QK^T)V is invariant
to a permutation of the key axis, so the result is exact.

Device algorithm (per core), bf16 matmuls with f32 PSUM accumulation:
  - x^T arrives pre-transposed [512, 4096] (host layout prep), cast to bf16
  - Q^T[64, 2048], and a fused [V^T; K^T] projection [128, 4096]
  - V^T is PE-transposed into V-natural [k,64] tiles with a ones column
    appended -> [k, 65]
  - scores are computed TRANSPOSED: score^T[k, q] blocks via
    lhsT=K^T-block (contraction=64).  Pairs of k-blocks are row-packed
    onto partition groups 0-63 / 64-127 so two matmuls run concurrently.
  - exp on the scalar engine straight out of PSUM (scale=1/8 fused)
  - P^T @ [V|1] accumulates Z^T[64, q] AND the softmax denominator
    (row 64) in one PSUM tile across all 32 k-blocks
  - reciprocal + rank-1 broadcast matmul + elementwise multiply
  - output is Z^T [64, 2048] f32; the host transposes back.
"""

import os
import sys

import numpy as np

for _p in ("/opt/trn_rl_repo", "/root/.axon_site/_ro/trn_rl_repo"):
    if os.path.isdir(_p) and _p not in sys.path:
        sys.path.insert(0, _p)

import concourse.bass as bass
import concourse.mybir as mybir
from concourse import bacc
from concourse.bass_utils import run_bass_kernel_spmd
from concourse.masks import make_identity
from concourse.tile import TileContext

F32 = mybir.dt.float32
BF16 = mybir.dt.bfloat16

B = 4          # batch
S = 4096       # sequence (keys)
SQ = 2048      # queries per core
W = 512        # d_model
E = 64         # d_head
P = 128
WC = W // P    # 4 w-chunks
NQC = SQ // 512  # 4 query chunks of 512
NKB = S // P   # 32 key blocks of 128
NSC = S // 512  # 8 chunks for the KV projection

N_CORES = 8


def build_graph() -> bass.Bass:
    nc = bacc.Bacc(
        "TRN2", target_bir_lowering=False, debug=False, num_devices=N_CORES
    )

    xt_d = nc.declare_dram_parameter("xt", [W, S], F32, isOutput=False)
    wq_d = nc.declare_dram_parameter("wq", [W, E], F32, isOutput=False)
    # wvk packs [Wv | Wk] -> [512, 128]
    wvk_d = nc.declare_dram_parameter("wvk", [W, 2 * E], F32, isOutput=False)
    bq_d = nc.declare_dram_parameter("bq", [E], F32, isOutput=False)
    # bkv packs [bv; bk] -> [128]
    bkv_d = nc.declare_dram_parameter("bkv", [2 * E], F32, isOutput=False)
    out_d = nc.declare_dram_parameter("out", [E, SQ], F32, isOutput=True)

    xt_view = xt_d.rearrange("(c p) s -> c p s", p=P)

    with TileContext(nc) as tc:
        with (
            tc.tile_pool(name="consts", bufs=1) as consts,
            tc.tile_pool(name="persist", bufs=1) as persist,
        ):
            # --- constants ---
            id64 = consts.tile([E, E], BF16)
            make_identity(nc, id64)
            onesw = consts.tile([E + 1, E], F32)
            nc.gpsimd.memset(onesw[E : E + 1, :], 1.0)
            bq_t = consts.tile([E, 1], F32)
            nc.sync.dma_start(bq_t, bq_d[:, None])
            bkv_t = consts.tile([P, 1], F32)
            nc.sync.dma_start(bkv_t, bkv_d[:, None])

            # --- persistent activations ---
            xtb = persist.tile([P, WC, S], BF16)      # x^T bf16
            qt = persist.tile([P, SQ], BF16)          # Q^T duplicated on both halves
            kvt = persist.tile([P, S], BF16)          # rows 0:64 V^T, 64:128 K^T
            ktd = persist.tile([P, S], BF16)          # rows 0:64 K^T (copy)
            vnat = persist.tile([P, NKB, E + 1], BF16)  # V natural + ones col

            # --- phase A: load, cast, project ---
            with (
                tc.tile_pool(name="pa_sb", bufs=2) as pa_sb,
                tc.tile_pool(name="pa_ps", bufs=2, space="PSUM") as pa_ps,
                tc.tile_pool(name="pa_pst", bufs=2, space="PSUM") as pa_pst,
            ):
                # weights
                wqf = pa_sb.tile([P, WC, E], F32, tag="wqf")
                nc.sync.dma_start(wqf, wq_d.rearrange("(c p) e -> p c e", p=P))
                wq_b = consts.tile([P, WC, E], BF16)
                nc.vector.tensor_copy(wq_b, wqf)
                wvkf = pa_sb.tile([P, WC, 2 * E], F32, tag="wvkf")
                nc.sync.dma_start(wvkf, wvk_d.rearrange("(c p) e -> p c e", p=P))
                wvk_b = consts.tile([P, WC, 2 * E], BF16)
                nc.vector.tensor_copy(wvk_b, wvkf)

                nc.gpsimd.memset(vnat[:, :, E : E + 1], 1.0)

                # x^T load/cast and projections, streamed per 1024-column
                # stripe (qq-major) so the PE starts projecting as soon as
                # the first stripe lands and never drains between stripes.
                QTR = S // 4
                for qq in range(4):
                    qsl = slice(qq * QTR, (qq + 1) * QTR)
                    for wc in range(WC):
                        xf = pa_sb.tile([P, QTR], F32, tag=f"xf{wc}")
                        nc.sync.dma_start(xf, xt_view[wc, :, qsl])
                        r = (qq * WC + wc) % 4
                        if r == 3:
                            nc.gpsimd.tensor_copy(xtb[:, wc, qsl], xf)
                        elif r == 1:
                            nc.scalar.copy(xtb[:, wc, qsl], xf)
                        else:
                            nc.vector.tensor_copy(xtb[:, wc, qsl], xf)

                    # Q^T projection (+bias) for the two 512-col chunks of
                    # this stripe (only the first SQ columns are queries)
                    if qq < 2:
                        for qc in (2 * qq, 2 * qq + 1):
                            cs = slice(qc * 512, (qc + 1) * 512)
                            qp = pa_ps.tile([E, 512], F32, tag="qp")
                            for wc in range(WC):
                                nc.tensor.matmul(
                                    qp, wq_b[:, wc, :], xtb[:, wc, cs],
                                    start=(wc == 0), stop=(wc == WC - 1),
                                )
                            nc.vector.tensor_scalar_add(qt[0:E, cs], qp, bq_t)
                        nc.sync.dma_start(qt[E:P, qsl], qt[0:E, qsl])

                    # fused [V^T; K^T] projection (+biases) for this stripe
                    for sc in (2 * qq, 2 * qq + 1):
                        cs = slice(sc * 512, (sc + 1) * 512)
                        kvp = pa_ps.tile([P, 512], F32, tag="kvp")
                        for wc in range(WC):
                            nc.tensor.matmul(
                                kvp, wvk_b[:, wc, :], xtb[:, wc, cs],
                                start=(wc == 0), stop=(wc == WC - 1),
                            )
                        nc.vector.tensor_scalar_add(kvt[:, cs], kvp, bkv_t)
                    # K^T copy for the even (partition 0-63) score matmuls
                    nc.sync.dma_start(ktd[0:E, qsl], kvt[E:P, qsl])

                    # V natural [k, 64] (+ones col) via PE transpose
                    for kb in range(qq * 8, qq * 8 + 8):
                        vps = pa_pst.tile([P, E], BF16, tag="vps")
                        nc.tensor.transpose(
                            vps, kvt[0:E, kb * P : (kb + 1) * P], id64
                        )
                        nc.vector.tensor_copy(vnat[:, kb, 0:E], vps)

            # --- phase B: flash attention sweep ---
            # PSUM budget (8 banks): sp [128,3,512] x2 bufs = 6, zp-tag x2 = 2.
            # Division tails are software-pipelined one q-chunk behind the
            # matmul sweep so the PE never stalls on the DVE reciprocal.
            G = 3
            groups = []
            kb = 0
            while kb < NKB:
                groups.append(list(range(kb, min(kb + G, NKB))))
                kb += G
            with (
                tc.tile_pool(name="sp", bufs=2, space="PSUM") as spP,
                tc.tile_pool(name="zp", bufs=2, space="PSUM") as zpP,
                tc.tile_pool(name="pexp", bufs=3) as peP,
                tc.tile_pool(name="fin", bufs=2) as finP,
            ):
                def sweep(qc):
                    qs = slice(qc * 512, (qc + 1) * 512)
                    zp = zpP.tile([E + 1, 512], F32, tag="zp")
                    for grp in groups:
                        sp = spP.tile([P, G, 512], F32, tag="sp")
                        n = len(grp)
                        for j, kb in enumerate(grp):
                            # score^T block [128k, 512q]; even/odd k-blocks
                            # row-packed onto partition groups 0-63 / 64-127
                            if kb % 2 == 0:
                                lhs, rhs = ktd[0:E, kb * P : (kb + 1) * P], qt[0:E, qs]
                            else:
                                lhs, rhs = kvt[E:P, kb * P : (kb + 1) * P], qt[E:P, qs]
                            nc.tensor.matmul(
                                sp[:, j, :], lhs, rhs, start=True, stop=True
                            )
                        pe = peP.tile([P, G, 512], BF16, tag="pe")
                        nc.scalar.activation(
                            pe[:, :n, :], sp[:, :n, :],
                            mybir.ActivationFunctionType.Exp, scale=0.125,
                        )
                        for j, kb in enumerate(grp):
                            nc.tensor.matmul(
                                zp, vnat[:, kb, :], pe[:, j, :],
                                start=(kb == 0), stop=(kb == NKB - 1),
                            )
                    # pull the accumulator out of PSUM right away to free
                    # the zp slot; the rest of the tail is deferred.
                    zsb = finP.tile([E + 1, 512], F32, tag="zsb")
                    nc.vector.tensor_copy(zsb, zp)
                    return zsb

                def tail(qc, zsb, last=False):
                    qs = slice(qc * 512, (qc + 1) * 512)
                    rdt = finP.tile([E + 1, 512], F32, tag="rdt")
                    if last:
                        # 1/d = exp(-log d) on ACT: ~1.2us vs 3.3us DVE
                        # reciprocal; only worth it for the final tail where
                        # nothing overlaps the latency.
                        lg = finP.tile([E + 1, 512], F32, tag="lg")
                        nc.scalar.activation(
                            lg[E : E + 1, :], zsb[E : E + 1, :],
                            mybir.ActivationFunctionType.Ln,
                        )
                        nc.scalar.activation(
                            rdt[E : E + 1, :], lg[E : E + 1, :],
                            mybir.ActivationFunctionType.Exp, scale=-1.0,
                        )
                    else:
                        nc.vector.reciprocal(rdt[E : E + 1, :], zsb[E : E + 1, :])
                    bc = zpP.tile([E + 1, 512], F32, tag="zp")
                    nc.tensor.matmul(
                        bc[0:E, :], onesw[E : E + 1, :], rdt[E : E + 1, :],
                        start=True, stop=True,
                    )
                    bcs = finP.tile([E, 512], F32, tag="bcs")
                    nc.vector.tensor_copy(bcs, bc[0:E, :])
                    zf = finP.tile([E, 512], F32, tag="zf")
                    nc.vector.tensor_tensor(
                        zf, zsb[0:E, :], bcs, mybir.AluOpType.mult
                    )
                    nc.sync.dma_start(out_d[:, qs], zf)

                pending = None
                for qc in range(NQC):
                    zsb = sweep(qc)
                    if pending is not None:
                        tail(*pending)
                    pending = (qc, zsb)
                tail(*pending, last=True)

    nc.compile()
    return nc


_GRAPH_CACHE: bass.Bass | None = None


def _get_graph() -> bass.Bass:
    global _GRAPH_CACHE
    if _GRAPH_CACHE is None:
        _GRAPH_CACHE = build_graph()
    return _GRAPH_CACHE


def _make_in_maps(x, Wq, bq, Wk, bk, Wv, bv):
    x = np.asarray(x, dtype=np.float32)
    wq = np.ascontiguousarray(np.asarray(Wq, dtype=np.float32))
    wvk = np.ascontiguousarray(
        np.concatenate(
            [np.asarray(Wv, dtype=np.float32), np.asarray(Wk, dtype=np.float32)],
            axis=1,
        )
    )
    bq_ = np.ascontiguousarray(np.asarray(bq, dtype=np.float32))
    bkv = np.ascontiguousarray(
        np.concatenate(
            [np.asarray(bv, dtype=np.float32), np.asarray(bk, dtype=np.float32)]
        )
    )
    in_maps = []
    for c in range(N_CORES):
        b, h = divmod(c, 2)
        xl = np.roll(x[b], -h * SQ, axis=0)
        xt = np.ascontiguousarray(xl.T)
        in_maps.append(
            {"xt": xt, "wq": wq, "wvk": wvk, "bq": bq_, "bkv": bkv}
        )
    return in_maps


def _run(inputs: dict, trace: bool = False):
    nc = _get_graph()
    in_maps = _make_in_maps(**inputs)
    res = run_bass_kernel_spmd(
        nc, in_maps, core_ids=list(range(N_CORES)), trace=trace
    )
    out = np.zeros((B, S, E), dtype=np.float32)
    for c in range(N_CORES):
        b, h = divmod(c, 2)
        out[b, h * SQ : (h + 1) * SQ, :] = res.results[c]["out"].T
    return out, res


def kernel(**inputs) -> np.ndarray:
    out, _ = _run(inputs, trace=False)
    return out


# revision 17
# speedup vs baseline: 1.1724x; 1.1724x over previous
"""Attention kernel for Trainium2, SPMD across 8 NeuronCores.

Problem: x[4, 4096, 512]; Q,K,V = x@W* + b* (d_head=64);
Z = softmax(Q K^T / 8) V  -> [4, 4096, 64]

Sharding: data-parallel over batch (4) x query-halves (2) = 8 cores.
Each core handles 2048 queries of one batch against all 4096 keys of
that batch.  The key/value rows are fed in rolled order so every core's
queries sit at rows 0..2047 of its input -- softmax(QK^T)V is invariant
to a permutation of the key axis, so the result is exact.

Device algorithm (per core), bf16 matmuls with f32 PSUM accumulation:
  - x^T arrives pre-transposed [512, 4096] (host layout prep), loaded in
    four 1024-column stripes (DMAs split across both HWDGE engines),
    cast to bf16 on DVE/Pool
  - per stripe: Q^T projection (stripes 0-1 only), fused [V^T; K^T]
    projection, V^T -> V-natural PE transposes (+ones column), and the
    flash sweep for query chunks 0-1 over that stripe's key blocks --
    so TensorE and ScalarE start working ~15us in, while later stripes
    are still loading
  - scores are computed TRANSPOSED: score^T[k, q] blocks with
    lhsT=K^T-block (contraction=64); even/odd key blocks are row-packed
    onto partition groups 0-63 / 64-127 so pairs run concurrently
  - exp on the scalar engine straight out of PSUM ([128, 2, 512] groups,
    scale=1/8 fused)
  - P^T @ [V|1] accumulates Z^T[64, q] AND the softmax denominator
    (row 64) in one PSUM tile across all 32 key blocks
  - query chunks 2-3 sweep after the stripes (everything resident)
  - division tails (reciprocal + rank-1 broadcast matmul + multiply) are
    software-pipelined so the PE never stalls on them
  - output is Z^T [64, 2048] f32; the host transposes back.
"""

import os
import sys

import numpy as np

for _p in ("/opt/trn_rl_repo", "/root/.axon_site/_ro/trn_rl_repo"):
    if os.path.isdir(_p) and _p not in sys.path:
        sys.path.insert(0, _p)

import concourse.bass as bass
import concourse.mybir as mybir
from concourse import bacc
from concourse.bass_utils import run_bass_kernel_spmd
from concourse.masks import make_identity
from concourse.tile import TileContext

F32 = mybir.dt.float32
BF16 = mybir.dt.bfloat16

B = 4          # batch
S = 4096       # sequence (keys)
SQ = 2048      # queries per core
W = 512        # d_model
E = 64         # d_head
P = 128
WC = W // P    # 4 w-chunks
NQC = SQ // 512  # 4 query chunks of 512
NKB = S // P   # 32 key blocks of 128
G = 2          # key blocks per exp group

N_CORES = 8


def build_graph() -> bass.Bass:
    nc = bacc.Bacc(
        "TRN2",
        target_bir_lowering=False,
        debug=False,
        num_devices=N_CORES,
        enable_partition_id=False,
        num_swdge_queues=2,
    )

    xt_d = nc.declare_dram_parameter("xt", [W, S], F32, isOutput=False)
    wq_d = nc.declare_dram_parameter("wq", [W, E], F32, isOutput=False)
    # wvk packs [Wv | Wk] -> [512, 128]
    wvk_d = nc.declare_dram_parameter("wvk", [W, 2 * E], F32, isOutput=False)
    bq_d = nc.declare_dram_parameter("bq", [E], F32, isOutput=False)
    # bkv packs [bv; bk] -> [128]
    bkv_d = nc.declare_dram_parameter("bkv", [2 * E], F32, isOutput=False)
    out_d = nc.declare_dram_parameter("out", [E, SQ], F32, isOutput=True)

    xt_view = xt_d.rearrange("(c p) s -> c p s", p=P)
    QTR = S // 4  # stripe width (1024 cols)

    with TileContext(nc) as tc:
        with (
            tc.tile_pool(name="consts", bufs=1) as consts,
            tc.tile_pool(name="persist", bufs=1) as persist,
            tc.tile_pool(name="stage", bufs=2) as stage,
            tc.tile_pool(name="pa", bufs=2, space="PSUM") as paP,
            tc.tile_pool(name="sp", bufs=2, space="PSUM") as spP,
            tc.tile_pool(name="zp", bufs=2, space="PSUM") as zpP,
            tc.tile_pool(name="pexp", bufs=4) as peP,
            tc.tile_pool(name="fin", bufs=2) as finP,
        ):
            # --- constants ---
            id64 = consts.tile([E, E], BF16)
            make_identity(nc, id64)
            onesw = consts.tile([E + 1, E], F32)
            nc.gpsimd.memset(onesw[E : E + 1, :], 1.0)
            bq_t = consts.tile([E, 1], F32)
            nc.sync.dma_start(bq_t, bq_d[:, None])
            bkv_t = consts.tile([P, 1], F32)
            nc.sync.dma_start(bkv_t, bkv_d[:, None])

            # --- persistent activations ---
            xtb = persist.tile([P, WC, S], BF16)      # x^T bf16
            qt = persist.tile([P, SQ], BF16)          # Q^T on both halves
            kvt = persist.tile([P, S], BF16)          # 0:64 V^T, 64:128 K^T
            ktd = persist.tile([P, S], BF16)          # 0:64 K^T (copy)
            vnat = persist.tile([P, NKB, E + 1], BF16)  # V natural + ones
            nc.gpsimd.memset(vnat[:, :, E : E + 1], 1.0)

            # weights (sync queue, small)
            wqf = stage.tile([P, WC, E], F32, tag="wqf")
            nc.sync.dma_start(wqf, wq_d.rearrange("(c p) e -> p c e", p=P))
            wq_b = consts.tile([P, WC, E], BF16)
            nc.vector.tensor_copy(wq_b, wqf)
            wvkf = stage.tile([P, WC, 2 * E], F32, tag="wvkf")
            nc.sync.dma_start(wvkf, wvk_d.rearrange("(c p) e -> p c e", p=P))
            wvk_b = consts.tile([P, WC, 2 * E], BF16)
            nc.vector.tensor_copy(wvk_b, wvkf)

            zps = {}

            def sweep_part(qc, g0, g1):
                """Score+exp+PV for query chunk qc, exp-groups [g0, g1)."""
                qs = slice(qc * 512, (qc + 1) * 512)
                if qc not in zps:
                    zps[qc] = zpP.tile(
                        [E + 1, 512], F32, tag="zp", name=f"zpacc{qc}"
                    )
                zp = zps[qc]
                for g in range(g0, g1):
                    kbs = list(range(g * G, min((g + 1) * G, NKB)))
                    sp = spP.tile([P, G, 512], F32, tag="sp")
                    for j, kb in enumerate(kbs):
                        if kb % 2 == 0:
                            lhs = ktd[0:E, kb * P : (kb + 1) * P]
                            rhs = qt[0:E, qs]
                        else:
                            lhs = kvt[E:P, kb * P : (kb + 1) * P]
                            rhs = qt[E:P, qs]
                        nc.tensor.matmul(
                            sp[:, j, :], lhs, rhs, start=True, stop=True
                        )
                    n = len(kbs)
                    pe = peP.tile([P, G, 512], BF16, tag="pe")
                    nc.scalar.activation(
                        pe[:, :n, :], sp[:, :n, :],
                        mybir.ActivationFunctionType.Exp, scale=0.125,
                    )
                    for j, kb in enumerate(kbs):
                        nc.tensor.matmul(
                            zp, vnat[:, kb, :], pe[:, j, :],
                            start=(kb == 0), stop=(kb == NKB - 1),
                        )

            def finish_sweep(qc):
                # pull Z^T+denom out of PSUM immediately to free the slot
                zsb = finP.tile([E + 1, 512], F32, tag="zsb")
                nc.vector.tensor_copy(zsb, zps[qc])
                del zps[qc]
                return zsb

            def tail(qc, zsb):
                qs = slice(qc * 512, (qc + 1) * 512)
                rdt = finP.tile([E + 1, 512], F32, tag="rdt")
                nc.vector.reciprocal(rdt[E : E + 1, :], zsb[E : E + 1, :])
                bc = zpP.tile([E + 1, 512], F32, tag="zp")
                nc.tensor.matmul(
                    bc[0:E, :], onesw[E : E + 1, :], rdt[E : E + 1, :],
                    start=True, stop=True,
                )
                bcs = finP.tile([E, 512], F32, tag="bcs")
                nc.vector.tensor_copy(bcs, bc[0:E, :])
                zf = finP.tile([E, 512], F32, tag="zf")
                nc.vector.tensor_tensor(
                    zf, zsb[0:E, :], bcs, mybir.AluOpType.mult
                )
                nc.gpsimd.dma_start(out_d[:, qs], zf)

            # --- streamed stripes: load, cast, project, sweep qc 0-1 ---
            for qq in range(4):
                qsl = slice(qq * QTR, (qq + 1) * QTR)
                for wc in range(WC):
                    xf = stage.tile([P, QTR], F32, tag=f"xf{wc}")
                    dma_eng = nc.sync if wc % 2 == 0 else nc.scalar
                    dma_eng.dma_start(xf, xt_view[wc, :, qsl])
                    # Pool casts measured ~4x slower than DVE; give it one
                    cast_eng = nc.gpsimd if wc == 3 else nc.vector
                    cast_eng.tensor_copy(xtb[:, wc, qsl], xf)

                if qq < 2:
                    for qc in (2 * qq, 2 * qq + 1):
                        cs = slice(qc * 512, (qc + 1) * 512)
                        qp = paP.tile([P, 512], F32, tag="pa")
                        for wc in range(WC):
                            nc.tensor.matmul(
                                qp[0:E, :], wq_b[:, wc, :], xtb[:, wc, cs],
                                start=(wc == 0), stop=(wc == WC - 1),
                            )
                        nc.vector.tensor_scalar_add(qt[0:E, cs], qp[0:E, :], bq_t)
                    nc.gpsimd.dma_start(qt[E:P, qsl], qt[0:E, qsl])

                for sc in (2 * qq, 2 * qq + 1):
                    cs = slice(sc * 512, (sc + 1) * 512)
                    kvp = paP.tile([P, 512], F32, tag="pa")
                    for wc in range(WC):
                        nc.tensor.matmul(
                            kvp, wvk_b[:, wc, :], xtb[:, wc, cs],
                            start=(wc == 0), stop=(wc == WC - 1),
                        )
                    nc.vector.tensor_scalar_add(kvt[:, cs], kvp, bkv_t)
                nc.gpsimd.dma_start(ktd[0:E, qsl], kvt[E:P, qsl])

                for kb in range(qq * 8, qq * 8 + 8):
                    vps = paP.tile([P, E], BF16, tag="pa")
                    nc.tensor.transpose(
                        vps, kvt[0:E, kb * P : (kb + 1) * P], id64
                    )
                    nc.vector.tensor_copy(vnat[:, kb, 0:E], vps)

                # sweep query chunks 0-1 over this stripe's key blocks
                gpq = 8 // G  # exp groups per stripe per qc
                for qc in (0, 1):
                    sweep_part(qc, qq * gpq, (qq + 1) * gpq)

            zsb0 = finish_sweep(0)
            zsb1 = finish_sweep(1)

            # --- back half: query chunks 2-3 (all data resident) ---
            NG = NKB // G
            sweep_part(2, 0, NG // 2)
            tail(0, zsb0)
            sweep_part(2, NG // 2, NG)
            zsb2 = finish_sweep(2)
            tail(1, zsb1)
            sweep_part(3, 0, NG // 2)
            tail(2, zsb2)
            sweep_part(3, NG // 2, NG)
            zsb3 = finish_sweep(3)
            tail(3, zsb3)

    nc.compile()
    return nc


_GRAPH_CACHE: bass.Bass | None = None


def _get_graph() -> bass.Bass:
    global _GRAPH_CACHE
    if _GRAPH_CACHE is None:
        _GRAPH_CACHE = build_graph()
    return _GRAPH_CACHE


def _make_in_maps(x, Wq, bq, Wk, bk, Wv, bv):
    x = np.asarray(x, dtype=np.float32)
    wq = np.ascontiguousarray(np.asarray(Wq, dtype=np.float32))
    wvk = np.ascontiguousarray(
        np.concatenate(
            [np.asarray(Wv, dtype=np.float32), np.asarray(Wk, dtype=np.float32)],
            axis=1,
        )
    )
    bq_ = np.ascontiguousarray(np.asarray(bq, dtype=np.float32))
    bkv = np.ascontiguousarray(
        np.concatenate(
            [np.asarray(bv, dtype=np.float32), np.asarray(bk, dtype=np.float32)]
        )
    )
    in_maps = []
    for c in range(N_CORES):
        b, h = divmod(c, 2)
        xl = np.roll(x[b], -h * SQ, axis=0)
        xt = np.ascontiguousarray(xl.T)
        in_maps.append({"xt": xt, "wq": wq, "wvk": wvk, "bq": bq_, "bkv": bkv})
    return in_maps


def _run(inputs: dict, trace: bool = False):
    nc = _get_graph()
    in_maps = _make_in_maps(**inputs)
    res = run_bass_kernel_spmd(
        nc, in_maps, core_ids=list(range(N_CORES)), trace=trace
    )
    out = np.zeros((B, S, E), dtype=np.float32)
    for c in range(N_CORES):
        b, h = divmod(c, 2)
        out[b, h * SQ : (h + 1) * SQ, :] = res.results[c]["out"].T
    return out, res


def kernel(**inputs) -> np.ndarray:
    out, _ = _run(inputs, trace=False)
    return out
